# revision 1
# baseline (speedup 1.0000x reference)
"""Trainium2 Bass kernel for a full attention block (QKV proj + RMSNorm + RoPE +
softmax attention + output proj), batch-data-parallel across 8 NeuronCores.

v2: bf16 datapath (inputs, weights, q/k/v, probabilities, out) with f32 psum
accumulation; transposes offloaded to DMA xbar (dma_start_transpose);
64-partition score matmuls via PE tile positioning (no zero-padded kz);
RMS factors applied via one stride-0-broadcast tensor op per tile; RMS
reductions on gpsimd.

Shapes (hardcoded): x (8, 1024, 1024), H=16 heads, hd=64.
Each core processes one batch element; weights are replicated.
"""
import numpy as np
import ml_dtypes

import concourse.bass as bass
from concourse import bacc
import concourse.mybir as mybir
import concourse.tile as tile
from concourse.bass_utils import run_bass_kernel_spmd

F32 = mybir.dt.float32
BF16 = mybir.dt.bfloat16
AF = mybir.ActivationFunctionType
ALU = mybir.AluOpType

B, L, C, H, HD = 8, 1024, 1024, 16, 64
EPS = 1e-6
NLB = L // 128   # 8 l-blocks
NCB = C // 128   # 8 c-blocks
NJB = L // 128   # 8 j-blocks
N_CORES = 8

_nc_cache = None
_last_results = None  # BassKernelResults of the most recent run (for test.py)


def _bcast(ap2d, reps):
    """(128, w) AP -> (128, reps, w) stride-0 broadcast view."""
    return bass.AP(tensor=ap2d.tensor, offset=ap2d.offset,
                   ap=[ap2d.ap[0], [0, reps], ap2d.ap[1]])


def _sub(ap3d, lo, w):
    """(128, reps, 64) bcast view -> free-dim slice [lo:lo+w]."""
    return bass.AP(tensor=ap3d.tensor, offset=ap3d.offset + lo,
                   ap=[ap3d.ap[0], ap3d.ap[1], [1, w]])


def _scalar_bcast(ap2d, w):
    """(128, n) AP -> (128, n, w) view broadcasting each scalar along w."""
    return bass.AP(tensor=ap2d.tensor, offset=ap2d.offset,
                   ap=[ap2d.ap[0], ap2d.ap[1], [0, w]])


def build_nc():
    nc = bacc.Bacc("TRN2", target_bir_lowering=False)

    xT = nc.declare_dram_parameter("xT", [C, L], BF16, isOutput=False)
    wq = nc.declare_dram_parameter("wq", [C, 3 * C], BF16, isOutput=False)
    wp = nc.declare_dram_parameter("wp", [C, C], BF16, isOutput=False)
    # RoPE tables with rms-norm weights folded in (host-prepared)
    cq = nc.declare_dram_parameter("cq", [L, HD], BF16, isOutput=False)
    sq = nc.declare_dram_parameter("sq", [L, HD], BF16, isOutput=False)
    ck = nc.declare_dram_parameter("ck", [L, HD], BF16, isOutput=False)
    sk = nc.declare_dram_parameter("sk", [L, HD], BF16, isOutput=False)
    y = nc.declare_dram_parameter("y", [L, C], F32, isOutput=True)

    def tab_view(t):
        # (L, 64) DRAM -> SBUF (128, 8, 64): element (p, lc, j) = t[128*lc + p, j]
        return bass.AP(tensor=t, offset=0,
                       ap=[[HD, 128], [128 * HD, NLB], [1, HD]])

    with tile.TileContext(nc) as tc:
        with tc.tile_pool(name="persist", bufs=1) as persist:
            # --- persistent tiles ---
            cq_sb = persist.tile([128, NLB, HD], BF16)
            sq_sb = persist.tile([128, NLB, HD], BF16)
            ck_sb = persist.tile([128, NLB, HD], BF16)
            sk_sb = persist.tile([128, NLB, HD], BF16)
            nc.sync.dma_start(out=cq_sb, in_=tab_view(cq))
            nc.sync.dma_start(out=sq_sb, in_=tab_view(sq))
            nc.sync.dma_start(out=ck_sb, in_=tab_view(ck))
            nc.sync.dma_start(out=sk_sb, in_=tab_view(sk))

            fk_all = persist.tile([128, NLB, H], F32)          # fk/8 per (j, head)
            vb = persist.tile([128, NLB, H, HD + 1], BF16)     # V blocks + ones col
            eps_q = persist.tile([128, 1], F32)
            nc.vector.memset(eps_q, EPS)
            eps_k = persist.tile([128, 1], F32)
            nc.vector.memset(eps_k, HD * EPS)

            zeros_b = persist.tile([HD, L], BF16)
            nc.vector.memset(zeros_b, 0.0)

            ones128f = persist.tile([128, 1], F32)
            nc.vector.memset(ones128f, 1.0)
            ones128 = persist.tile([128, 1], BF16)
            nc.vector.tensor_copy(ones128, ones128f)
            nc.vector.tensor_copy(
                bass.AP(tensor=vb.tensor, offset=vb.offset + HD,
                        ap=[vb.ap[0], vb.ap[1], vb.ap[2], [1, 1]]),
                bass.AP(tensor=ones128.tensor, offset=ones128.offset,
                        ap=[ones128.ap[0], [0, NLB], [0, H], [1, 1]]))

            # big persistent sbuf tensors (bf16, 16KB/partition each)
            xr = persist.tile([128, NCB, L], BF16)    # x^T: partition=c, free=l
            qr = persist.tile([128, NLB, C], BF16)    # q: partition=l, free=c
            kr = persist.tile([128, NLB, C], BF16)
            qT = persist.tile([128, NCB, L], BF16)    # q^T: partition=c, free=l
            kT = persist.tile([128, NCB, L], BF16)
            outT = persist.tile([128, NCB, L], BF16)  # attn out^T
            wpn = persist.tile([128, NCB, C], BF16)   # w_proj^T staged

            with tc.tile_pool(name="work", bufs=1) as p1, \
                 tc.tile_pool(name="psA", bufs=2, space="PSUM") as psA, \
                 tc.tile_pool(name="psO", bufs=2, space="PSUM") as psO:

                # element (p, cc, l) = xT[128*cc + p, l]; split halves so the
                # first chain can start after half the x transfer
                nc.sync.dma_start(out=xr[:, 0:4, :], in_=bass.AP(
                    tensor=xT, offset=0,
                    ap=[[L, 128], [128 * L, 4], [1, L]]))
                nc.sync.dma_start(out=xr[:, 4:8, :], in_=bass.AP(
                    tensor=xT, offset=4 * 128 * L,
                    ap=[[L, 128], [128 * L, 4], [1, L]]))

                def _sl4(ap4, lo, w):
                    return bass.AP(tensor=ap4.tensor, offset=ap4.offset + lo,
                                   ap=[ap4.ap[0], ap4.ap[1], ap4.ap[2], [1, w]])

                def rope16(stf2, dst2, cos_sb, sin_sb, lbp):
                    # stf2/dst2: (128, 2, 8, 64) bf16 covering l-blocks lbp, lbp+1
                    def tab2(t):
                        return bass.AP(
                            tensor=t.tensor, offset=t.offset + lbp * HD,
                            ap=[t.ap[0], [HD, 2], [0, 8], [1, HD]])
                    cw = tab2(cos_sb)
                    sw = tab2(sin_sb)
                    a_t = p1.tile([128, 2, 8, HD], BF16, tag="ropeA2", bufs=2)
                    nc.vector.tensor_mul(a_t, stf2, cw)
                    b_t = p1.tile([128, 2, 8, HD], BF16, tag="ropeB2", bufs=2)
                    nc.vector.tensor_mul(_sl4(b_t, 0, 32), _sl4(stf2, 32, 32),
                                         _sl4(sw, 0, 32))
                    nc.vector.tensor_mul(_sl4(b_t, 32, 32), _sl4(stf2, 0, 32),
                                         _sl4(sw, 32, 32))
                    nc.vector.tensor_add(dst2, a_t, b_t)

                def rope8(st3, dst3, cos_sb, sin_sb, lb):
                    # st3 (128, 8, 64) bf16; dst3 (128, 8, 64) bf16 slice
                    cw = _bcast(cos_sb[:, lb, :], 8)
                    sw = _bcast(sin_sb[:, lb, :], 8)
                    a_t = p1.tile([128, 8, HD], BF16, tag="ropeA", bufs=2)
                    nc.vector.tensor_mul(a_t, st3, cw)
                    b_t = p1.tile([128, 8, HD], BF16, tag="ropeB", bufs=2)
                    nc.vector.tensor_mul(b_t[:, :, 0:32], st3[:, :, 32:64],
                                         _sub(sw, 0, 32))
                    nc.vector.tensor_mul(b_t[:, :, 32:64], st3[:, :, 0:32],
                                         _sub(sw, 32, 32))
                    nc.vector.tensor_add(dst3, a_t, b_t)

                # ---------------- phase 1: qkv + rms + rope ----------------
                with nc.named_scope("qkv"):
                    for n in (0, 2, 4, 1, 3, 5):
                        wqn = p1.tile([128, NCB, 512], BF16, tag="wqn", bufs=2)
                        # element (p, cc, j) = wq[128*cc + p, 512*n + j]
                        nc.sync.dma_start(out=wqn, in_=bass.AP(
                            tensor=wq, offset=512 * n,
                            ap=[[3 * C, 128], [128 * 3 * C, NCB], [1, 512]]))

                        for lbp in range(0, NLB, 2):
                            ps = psA.tile([128, 1024], F32, tag="A")
                            for hhalf in range(2):
                                lb = lbp + hhalf
                                for cb in range(NCB):
                                    nc.tensor.matmul(
                                        ps[:, 512 * hhalf:512 * (hhalf + 1)],
                                        lhsT=xr[:, cb, 128 * lb:128 * (lb + 1)],
                                        rhs=wqn[:, cb, :],
                                        start=(cb == 0), stop=(cb == NCB - 1))
                            if n < 4:  # q or k: stats+fold both halves, one wide rope
                                stf2 = p1.tile([128, 2, 8, HD], BF16, tag="stf2",
                                               bufs=2)
                                for hhalf in range(2):
                                    lb = lbp + hhalf
                                    psh = ps[:, 512 * hhalf:512 * (hhalf + 1)]
                                    ps3 = psh.rearrange("p (h d) -> p h d", d=HD)
                                    sqt = p1.tile([128, 512], BF16, tag="sqt",
                                                  bufs=2)
                                    nc.scalar.activation(sqt, psh, AF.Square)
                                    sst = p1.tile([128, 8], F32, tag="sst", bufs=3)
                                    nc.vector.tensor_reduce(
                                        sst, sqt.rearrange("p (h d) -> p h d", d=HD),
                                        axis=mybir.AxisListType.X, op=ALU.add)
                                    rt = p1.tile([128, 8], F32, tag="rt", bufs=2)
                                    if n < 2:
                                        nc.scalar.activation(rt, sst, AF.Sqrt,
                                                             scale=1.0 / HD,
                                                             bias=eps_q)
                                        fv = p1.tile([128, 8], F32, tag="fqv",
                                                     bufs=2)
                                        nc.vector.reciprocal_approx_fast(fv, rt)
                                    else:
                                        nc.scalar.activation(rt, sst, AF.Sqrt,
                                                             scale=1.0, bias=eps_k)
                                        nc.vector.reciprocal_approx_fast(
                                            fk_all[:, lb,
                                                   8 * (n - 2):8 * (n - 1)],
                                            rt)
                                    if n < 2:
                                        st = p1.tile([128, 8, HD], BF16,
                                                     tag="st", bufs=2)
                                        nc.scalar.copy(st, ps3)
                                        nc.vector.tensor_mul(
                                            stf2[:, hhalf], st,
                                            _scalar_bcast(fv, HD))
                                    else:
                                        nc.scalar.copy(stf2[:, hhalf], ps3)
                                if n < 2:
                                    dst2 = qr[:, lbp:lbp + 2,
                                              512 * n:512 * (n + 1)].rearrange(
                                        "p u (h d) -> p u h d", d=HD)
                                    rope16(stf2, dst2, cq_sb, sq_sb, lbp)
                                    for hhalf in range(2):
                                        lb = lbp + hhalf
                                        nc.sync.dma_start_transpose(
                                            out=qT[:, 4 * n:4 * (n + 1),
                                                   128 * lb:128 * (lb + 1)],
                                            in_=qr[:, lb, 512 * n:512 * (n + 1)])
                                else:
                                    dst2 = kr[:, lbp:lbp + 2,
                                              512 * (n - 2):512 * (n - 1)].rearrange(
                                        "p u (h d) -> p u h d", d=HD)
                                    rope16(stf2, dst2, ck_sb, sk_sb, lbp)
                                    for hhalf in range(2):
                                        lb = lbp + hhalf
                                        nc.sync.dma_start_transpose(
                                            out=kT[:, 4 * (n - 2):4 * (n - 1),
                                                   128 * lb:128 * (lb + 1)],
                                            in_=kr[:, lb, 512 * (n - 2):512 * (n - 1)])
                            else:          # v heads 8(n-4) ..
                                for hhalf in range(2):
                                    lb = lbp + hhalf
                                    psh = ps[:, 512 * hhalf:512 * (hhalf + 1)]
                                    ps3 = psh.rearrange("p (h d) -> p h d", d=HD)
                                    nc.scalar.copy(
                                        vb[:, lb, 8 * (n - 4):8 * (n - 3), 0:HD],
                                        ps3)

                # ---------------- phase 3: attention ----------------
                with nc.named_scope("attn"):
                    nc.sync.dma_start(out=wpn, in_=bass.AP(
                        tensor=wp, offset=0,
                        ap=[[C, 128], [128 * C, NCB], [1, C]]))
                    kz_tiles = {}

                    def make_kz(hc):
                        for i in range(2):
                            h = 2 * hc + i
                            hp = 64 * i
                            kz = p1.tile([128, L], BF16, tag="kz", bufs=3,
                                         name=f"kz_{h}")
                            nc.vector.tensor_copy(kz[hp:hp + HD, :],
                                                  kT[hp:hp + HD, hc, :])
                            nc.vector.tensor_copy(
                                kz[64 * (1 - i):64 * (1 - i) + HD, :],
                                zeros_b)
                            kz_tiles[h] = kz

                    for hc in range(H // 2):
                        if hc == 0:
                            make_kz(0)
                        if hc + 1 < H // 2:
                            make_kz(hc + 1)
                        for i in range(2):
                            h = 2 * hc + i
                            hp = 64 * i
                            kz = kz_tiles.pop(h)
                            pso = psO.tile([HD + 1, L], F32, tag="O",
                                           name=f"pso_{h}")
                            for jb in range(NJB):
                                sts = psA.tile([128, L], F32, tag="A",
                                               name=f"st_{h}_{jb}")
                                for hf in range(2):
                                    nc.tensor.matmul(
                                        sts[:, 512 * hf:512 * (hf + 1)],
                                        lhsT=kz[:, 128 * jb:128 * (jb + 1)],
                                        rhs=qT[:, hc,
                                               512 * hf:512 * (hf + 1)],
                                        start=True, stop=True)
                                pt = p1.tile([128, L], BF16, tag="pt", bufs=3,
                                             name=f"pt_{h}_{jb}")
                                nc.scalar.activation(pt, sts, AF.Exp,
                                                     scale=fk_all[:, jb, h:h + 1])
                                for hf in range(2):
                                    nc.tensor.matmul(
                                        pso[:, 512 * hf:512 * (hf + 1)],
                                        lhsT=vb[:, jb, h, :],
                                        rhs=pt[:, 512 * hf:512 * (hf + 1)],
                                        start=(jb == 0), stop=(jb == NJB - 1))
                            srow = p1.tile([1, L], F32, tag="srow", bufs=2)
                            nc.vector.tensor_copy(srow, pso[HD:HD + 1, :])
                            rs = p1.tile([1, L], F32, tag="rs", bufs=2)
                            nc.vector.reciprocal_approx_fast(rs, srow)
                            fsb = p1.tile([HD, L], F32, tag="fsb", bufs=2)
                            nc.gpsimd.partition_broadcast(fsb, rs)
                            nc.vector.tensor_mul(
                                outT[hp:hp + HD, hc, :], pso[0:HD, :], fsb)

                # ---------------- phase 4: output projection ----------------
                with nc.named_scope("proj"):
                    for lb in range(NLB):
                        psy = psA.tile([128, 1024], F32, tag="A",
                                       name=f"psy_{lb}")
                        for hf in range(2):
                            for cb in range(NCB):
                                nc.tensor.matmul(
                                    psy[:, 512 * hf:512 * (hf + 1)],
                                    lhsT=outT[:, cb, 128 * lb:128 * (lb + 1)],
                                    rhs=wpn[:, cb, 512 * hf:512 * (hf + 1)],
                                    start=(cb == 0), stop=(cb == NCB - 1))
                        for half in range(2):
                            ysb = p1.tile([128, 512], F32, tag="ysb", bufs=2)
                            nc.vector.tensor_copy(
                                ysb, psy[:, 512 * half:512 * (half + 1)])
                            nc.sync.dma_start(
                                out=y[128 * lb:128 * (lb + 1),
                                      512 * half:512 * (half + 1)],
                                in_=ysb)

    nc.compile()
    return nc


def _get_nc():
    global _nc_cache
    if _nc_cache is None:
        _nc_cache = build_nc()
    return _nc_cache


def _host_prep(x, cos, sin, w_qkv, w_proj, q_norm_w, k_norm_w):
    bf16 = ml_dtypes.bfloat16
    x = np.asarray(x, dtype=np.float32)
    cos = np.asarray(cos, dtype=np.float32)
    sin = np.asarray(sin, dtype=np.float32)
    w_qkv = np.asarray(w_qkv, dtype=np.float32)
    w_proj = np.asarray(w_proj, dtype=np.float32)
    q_norm_w = np.asarray(q_norm_w, dtype=np.float32)
    k_norm_w = np.asarray(k_norm_w, dtype=np.float32)

    wqT = np.ascontiguousarray(w_qkv.T).astype(bf16)   # (C, 3C)
    wpT = np.ascontiguousarray(w_proj.T).astype(bf16)  # (C, C)

    def fold(w):
        # cosW[l,d] = cos[l,d]*w[d]
        # sinW[l,d<32] = -sin[l,d]*w[d+32]; sinW[l,d>=32] = sin[l,d]*w[d-32]
        cosW = cos * w[None, :]
        w_rot = np.concatenate([w[32:], w[:32]])
        sinW = (sin * w_rot[None, :]).copy()
        sinW[:, :32] *= -1.0
        return (np.ascontiguousarray(cosW).astype(bf16),
                np.ascontiguousarray(sinW).astype(bf16))

    cqt, sqt = fold(q_norm_w)
    ckt, skt = fold(k_norm_w)

    in_maps = []
    for b in range(N_CORES):
        in_maps.append({
            "xT": np.ascontiguousarray(x[b].T).astype(bf16),
            "wq": wqT, "wp": wpT,
            "cq": cqt, "sq": sqt, "ck": ckt, "sk": skt,
        })
    return in_maps


def kernel(x, cos, sin, w_qkv, w_proj, q_norm_w, k_norm_w, _trace=False):
    global _last_results
    nc = _get_nc()
    in_maps = _host_prep(x, cos, sin, w_qkv, w_proj, q_norm_w, k_norm_w)
    r = run_bass_kernel_spmd(nc, in_maps, list(range(N_CORES)), trace=_trace)
    _last_results = r
    return np.stack([r.results[b]["y"] for b in range(N_CORES)], axis=0)

